# revision 23
# baseline (speedup 1.0000x reference)
"""Trainium2 Bass kernel for nn_Attention_66640712565009 (topk_masking).

reference:
    a = relu(x0 @ W)                    # [B, T, 1], B=64, T=8192, D=128
    thr = min(top_k(a[..., 0], k))      # per batch row, k=25
    m = (a >= thr)
    e = exp(a) * m
    out = e / sum_T(e)                  # [B, T, 1]

Sharding: pure data parallel over batch: 8 rows per core across 8 cores.

Per-core layout: scores A [128 partitions, 512 free]; partition p = 16*r + j
(r = local row 0..7, j = T-block 0..15), free f in [0,512): t = j*512 + f.

Matvec runs as fused DVE scalar_tensor_tensor ops: (x_tile * Wb) with
accum_out giving the per-partition sum over D=128 in one pass.  Top-k uses
max8/match_replace knockout rounds per partition, then a tiny SBUF gather to
one-row-per-partition layout for the exact k-th value.  Row sums + broadcast
are one PE matmul with a block-diagonal 0/1 stationary.
"""

import numpy as np

B, T, D = 64, 8192, 128
N_CORES = 8
RPC = B // N_CORES          # rows per core = 8
NJ = 16                     # T-blocks per row (partitions per row)
FPT = T // NJ               # free elems per partition = 512
TS = 16                     # T-positions per partition per macro-tile
NT = FPT // TS              # macro-tiles = 32
NEG = -3.0e38               # knockout sentinel


def build(k: int, reps: int = 1, stt_split=None, mode="full", dma_ways=1,
          xbufs=16, sbufs=2, ts=TS, gps=(0, 1), gps_levels=7):
    """Build the per-core Bass program.  reps>1 wraps the body in a dynamic
    loop (for timing).  Returns compiled nc.

    stt_split: optional (dve, act, gps) integer tuple summing to TS,
    distributing the TS dot-product columns of each macro-tile across
    engines (DVE fused STT / DVE-mult+ACT-reduce / GPSIMD fused STT).
    Default: all on DVE.
    """
    import concourse.tile as tile
    from concourse import bacc, mybir

    f32 = mybir.dt.float32
    Alu = mybir.AluOpType
    Act = mybir.ActivationFunctionType

    k = int(k)
    assert 2 <= k <= 256
    ka_rounds = (k + 7) // 8          # knockout rounds for per-partition candidates
    kb_rounds = (k - 1) // 8          # full knockout rounds on the gathered row
    kb_rem = (k - 1) % 8              # remaining rank within the next max8

    nt = FPT // ts
    nc = bacc.Bacc("TRN2", target_bir_lowering=False, debug=False,
                   num_devices=N_CORES)
    x0 = nc.dram_tensor("x0", [RPC, T, D], f32, kind="ExternalInput").ap()
    wb_d = nc.dram_tensor("wb", [128, TS * D], f32, kind="ExternalInput").ap()
    sblk_d = nc.dram_tensor("sblk", [128, 128], f32, kind="ExternalInput").ap()
    out = nc.dram_tensor("out", [RPC, T], f32, kind="ExternalOutput").ap()

    # DRAM views. x0[r, j*512 + n*TS + f, d] -> partition (r j), tile n, free (f d)
    x0_v = x0.rearrange("r (j n f) d -> (r j) n (f d)", j=NJ, n=nt, f=ts)
    out_v = out.rearrange("r (j f) -> (r j) f", j=NJ)

    with tile.TileContext(nc) as tc:
        cpool = tc.alloc_tile_pool(name="consts", bufs=1)
        xpool = tc.alloc_tile_pool(name="xin", bufs=xbufs)
        spool = tc.alloc_tile_pool(name="scratch", bufs=sbufs)
        apool = tc.alloc_tile_pool(name="acc", bufs=2)
        ppool = tc.alloc_tile_pool(name="psum", bufs=2, space="PSUM")

        wb_cols = (TS if ((stt_split and stt_split[1]) or gps[0]) else 1) * D
        wb = cpool.tile([128, wb_cols], f32)
        nc.sync.dma_start(out=wb[:], in_=wb_d[:, 0:wb_cols])
        sblk = cpool.tile([128, 128], f32)
        nc.sync.dma_start(out=sblk[:], in_=sblk_d[:])

        def body():
            A = apool.tile([128, FPT], f32, tag="A")
            # ---- matvec: A[p, col] = sum_d x0_tile[p, col, d] * W[d] ----
            split = stt_split or (ts, 0, 0)
            dve_n, act_n, gps_n = split
            assert dve_n + act_n + gps_n == ts
            xt0 = None
            if mode == "compute":
                xt0 = xpool.tile([128, ts * D], f32, tag="xt")
                nc.sync.dma_start(out=xt0[:], in_=x0_v[:, 0, :])
            for n in range(nt):
                if mode == "compute":
                    xt = xt0
                else:
                    xt = xpool.tile([128, ts * D], f32, tag="xt")
                    if dma_ways == 1:
                        nc.sync.dma_start(out=xt[:], in_=x0_v[:, n, :])
                    else:
                        step = 128 // dma_ways
                        for w in range(dma_ways):
                            eng = nc.sync if w % 2 == 0 else nc.scalar
                            eng.dma_start(
                                out=xt[w * step:(w + 1) * step, :],
                                in_=x0_v[w * step:(w + 1) * step, n, :])
                if mode == "dma":
                    continue
                assert xt is not None
                # one big DVE multiply covering the ACT-reduced columns
                if act_n:
                    P = spool.tile([128, act_n * D], f32, tag="P")
                    nc.vector.tensor_tensor(
                        P[:], xt[:, dve_n * D:(dve_n + act_n) * D],
                        wb[:, 0:act_n * D], Alu.mult)
                scbig = spool.tile([128, ts * D], f32, tag="scbig")
                if gps[0] and (n % gps[1]) < gps[0]:
                    # DVE big multiply; GPSIMD log2 halving-add reduce
                    nc.vector.tensor_tensor(
                        scbig[:], xt[:], wb[:, 0:ts * D], Alu.mult)
                    sb3 = scbig[:].rearrange("p (t d) -> p t d", t=ts)
                    w_ = D // 2
                    for lv in range(gps_levels):
                        if w_ > 1:
                            nc.gpsimd.tensor_tensor(
                                sb3[:, :, 0:w_], sb3[:, :, 0:w_],
                                sb3[:, :, w_:2 * w_], Alu.add)
                            w_ //= 2
                    if w_ == 1:
                        nc.gpsimd.tensor_tensor(
                            A[:, n * ts:(n + 1) * ts],
                            sb3[:, :, 0:1].rearrange("p t d -> p (t d)"),
                            sb3[:, :, 1:2].rearrange("p t d -> p (t d)"),
                            Alu.add)
                    else:
                        nc.vector.tensor_reduce(
                            A[:, n * ts:(n + 1) * ts], sb3[:, :, 0:2 * w_],
                            axis=mybir.AxisListType.X, op=Alu.add)
                    continue
                for i in range(ts):
                    col = n * ts + i
                    if i < dve_n:
                        nc.vector.scalar_tensor_tensor(
                            scbig[:, i * D:(i + 1) * D],
                            xt[:, i * D:(i + 1) * D], 1.0, wb[:, 0:D],
                            Alu.mult, Alu.mult,
                            accum_out=A[:, col:col + 1])
                    elif i < dve_n + act_n:
                        j = i - dve_n
                        sc2 = spool.tile([128, D], f32, tag="sc2")
                        nc.scalar.activation(
                            sc2[:], P[:, j * D:(j + 1) * D], Act.Copy,
                            accum_out=A[:, col:col + 1])
                    else:
                        sc = spool.tile([128, D], f32, tag="sc")
                        nc.gpsimd.scalar_tensor_tensor(
                            sc[:], xt[:, i * D:(i + 1) * D], 1.0, wb[:, 0:D],
                            Alu.mult, Alu.mult,
                            accum_out=A[:, col:col + 1])

            if mode == "dma":
                O = apool.tile([128, FPT], f32, tag="O")
                nc.vector.memset(O[:], 0.0)
                nc.sync.dma_start(out=out_v[:, :], in_=O[:])
                return

            # ---- relu ----
            A2 = apool.tile([128, FPT], f32, tag="A2")
            nc.vector.tensor_scalar_max(A2[:], A[:], 0.0)

            if mode == "matvec":
                nc.sync.dma_start(out=out_v[:, :], in_=A2[:])
                return

            # ---- top-k phase A: per-partition top-(8*ka_rounds) ----
            C = apool.tile([128, FPT], f32, tag="C")
            nc.vector.tensor_copy(C[:], A2[:])
            cand = apool.tile([128, 8 * ka_rounds], f32, tag="cand")
            for rnd in range(ka_rounds):
                nc.vector.max(cand[:, 8 * rnd:8 * rnd + 8], C[:])
                if rnd + 1 < ka_rounds:
                    nc.vector.match_replace(
                        C[:], cand[:, 8 * rnd:8 * rnd + 8], C[:], NEG)

            # ---- gather candidates to one row per partition ----
            crow = apool.tile([RPC, NJ * 8 * ka_rounds], f32, tag="crow")
            for r in range(RPC):
                nc.sync.dma_start(
                    out=crow[r:r + 1, :],
                    in_=cand[16 * r:16 * r + 16, :])

            # ---- phase B: exact k-th largest of each row ----
            c8 = apool.tile([RPC, 8], f32, tag="c8")
            for rnd in range(kb_rounds):
                nc.vector.max(c8[:], crow[:])
                nc.vector.match_replace(crow[:], c8[:], crow[:], NEG)
            thr = apool.tile([RPC, 1], f32, tag="thr")
            if kb_rem == 0:
                nc.vector.tensor_reduce(thr[:], crow[:],
                                        axis=mybir.AxisListType.X, op=Alu.max)
            else:
                nc.vector.max(c8[:], crow[:])
                nc.vector.tensor_copy(thr[:], c8[:, kb_rem:kb_rem + 1])

            # ---- broadcast thr back to [128, 1] ----
            ones16 = cpool.tile([RPC, NJ], f32, tag="ones16")
            nc.vector.memset(ones16[:], 1.0)
            thr16 = apool.tile([RPC, NJ], f32, tag="thr16")
            nc.vector.tensor_scalar_mul(thr16[:], ones16[:], thr[:])
            thrp = apool.tile([128, 1], f32, tag="thrp")
            for r in range(RPC):
                nc.sync.dma_start(out=thrp[16 * r:16 * r + 16, :],
                                  in_=thr16[r:r + 1, :])

            # ---- mask, exp, fused multiply + per-partition sum ----
            M = apool.tile([128, FPT], f32, tag="M")
            nc.vector.tensor_scalar(M[:], A2[:], thrp[:, 0:1], None, Alu.is_ge)
            E = apool.tile([128, FPT], f32, tag="E")
            nc.scalar.activation(E[:], A2[:], Act.Exp)
            E2 = apool.tile([128, FPT], f32, tag="E2")
            psum = apool.tile([128, 1], f32, tag="psum")
            nc.vector.scalar_tensor_tensor(
                E2[:], E[:], 1.0, M[:], Alu.mult, Alu.mult,
                accum_out=psum[:])

            # ---- row sums broadcast via block-diagonal matmul ----
            rs = ppool.tile([128, 1], f32, tag="rs")
            nc.tensor.matmul(rs[:], sblk[:], psum[:], start=True, stop=True)
            rinv = apool.tile([128, 1], f32, tag="rinv")
            nc.vector.reciprocal(rinv[:], rs[:])

            # ---- normalize + store ----
            O = apool.tile([128, FPT], f32, tag="O")
            nc.vector.tensor_scalar_mul(O[:], E2[:], rinv[:, 0:1])
            nc.sync.dma_start(out=out_v[:, :], in_=O[:])

        if reps == 1:
            body()
        else:
            with tc.For_i(0, reps, 1):
                body()

        for p in (ppool, apool, spool, xpool, cpool):
            p.release()

    nc.compile()
    return nc


def _consts(W):
    wb = np.ascontiguousarray(
        np.tile(np.asarray(W, np.float32).reshape(1, D), (128, TS)))
    sblk = np.zeros((128, 128), np.float32)
    for r in range(RPC):
        sblk[16 * r:16 * r + 16, 16 * r:16 * r + 16] = 1.0
    return wb, sblk


_CACHE = {}


def kernel(x0, W, k):
    from concourse.bass_utils import run_bass_kernel_spmd

    k = int(np.asarray(k))
    x0 = np.ascontiguousarray(np.asarray(x0, dtype=np.float32))
    assert x0.shape == (B, T, D), x0.shape
    nc = _CACHE.get(k)
    if nc is None:
        nc = _CACHE[k] = build(k)
    wb, sblk = _consts(W)
    in_maps = [
        {"x0": x0[c * RPC:(c + 1) * RPC], "wb": wb, "sblk": sblk}
        for c in range(N_CORES)
    ]
    res = run_bass_kernel_spmd(nc, in_maps, core_ids=list(range(N_CORES)))
    full = np.concatenate([res.results[c]["out"] for c in range(N_CORES)], axis=0)
    return full.reshape(B, T, 1).astype(np.float32)


# revision 24
# speedup vs baseline: 1.0304x; 1.0304x over previous
"""Trainium2 Bass kernel for nn_Attention_66640712565009 (topk_masking).

reference:
    a = relu(x0 @ W)                    # [B, T, 1], B=64, T=8192, D=128
    thr = min(top_k(a[..., 0], k))      # per batch row, k=25
    m = (a >= thr)
    e = exp(a) * m
    out = e / sum_T(e)                  # [B, T, 1]

Sharding: pure data parallel over batch: 8 rows per core across 8 cores.
No collectives; per-core outputs are concatenated on the host.

Per-core layout: scores A [128 partitions, 512 free]; partition p = 16*r + j
(r = local row 0..7, j = T-block 0..15), free f in [0,512): t = j*512 + f.
x0 streams HBM->SBUF in 32 macro-tiles of [128, 16*128] (8KB contiguous per
partition, full DMA bandwidth), 16-deep buffered.

The matvec is fused DVE scalar_tensor_tensor ops: (x_col * W_bcast) with
accum_out giving each per-partition dot over D=128 in one pass — no
transpose anywhere (fp32 DMA-transpose doesn't exist on trn2, and TensorE
contracts over the partition dim, which x0's natural layout can't feed).
Top-k: max8/match_replace knockout rounds per partition -> 512 candidates
per row -> tiny SBUF gather to one-row-per-partition -> 3 more knockout
rounds + reduce-max for the exact 25th value.  Row softmax sums + broadcast
are one PE matmul with a block-diagonal 0/1 stationary.

Measured (8-core TRN2, reps-delta timing): ~120-150us/iter depending on
device load; DMA-only floor ~78us, all-DVE compute dominates.  Rel err vs
the jax reference: ~8e-7.

Knobs kept from the optimization search (defaults are the tuned champion):
stt_split/gps route columns to ScalarE/GPSIMD — both measured slower than
the pure-DVE fused path (ACT ~620ns per 128-col reduce; Pool rejects
TensorScalarPtr and its tensor_tensor has ~1us/op dispatch overhead).
"""

import numpy as np

B, T, D = 64, 8192, 128
N_CORES = 8
RPC = B // N_CORES          # rows per core = 8
NJ = 16                     # T-blocks per row (partitions per row)
FPT = T // NJ               # free elems per partition = 512
TS = 16                     # T-positions per partition per macro-tile
NT = FPT // TS              # macro-tiles = 32
NEG = -3.0e38               # knockout sentinel


def build(k: int, reps: int = 1, stt_split=None, mode="full", dma_ways=1,
          xbufs=16, sbufs=2, ts=TS, gps=(0, 1), gps_levels=7):
    """Build the per-core Bass program.  reps>1 wraps the body in a dynamic
    loop (for timing).  Returns compiled nc.

    stt_split: optional (dve, act, gps) integer tuple summing to TS,
    distributing the TS dot-product columns of each macro-tile across
    engines (DVE fused STT / DVE-mult+ACT-reduce / GPSIMD fused STT).
    Default: all on DVE.
    """
    import concourse.tile as tile
    from concourse import bacc, mybir

    f32 = mybir.dt.float32
    Alu = mybir.AluOpType
    Act = mybir.ActivationFunctionType

    k = int(k)
    assert 2 <= k <= 256
    ka_rounds = (k + 7) // 8          # knockout rounds for per-partition candidates
    kb_rounds = (k - 1) // 8          # full knockout rounds on the gathered row
    kb_rem = (k - 1) % 8              # remaining rank within the next max8

    nt = FPT // ts
    nc = bacc.Bacc("TRN2", target_bir_lowering=False, debug=False,
                   num_devices=N_CORES)
    x0 = nc.dram_tensor("x0", [RPC, T, D], f32, kind="ExternalInput").ap()
    wb_d = nc.dram_tensor("wb", [128, TS * D], f32, kind="ExternalInput").ap()
    sblk_d = nc.dram_tensor("sblk", [128, 128], f32, kind="ExternalInput").ap()
    out = nc.dram_tensor("out", [RPC, T], f32, kind="ExternalOutput").ap()

    # DRAM views. x0[r, j*512 + n*TS + f, d] -> partition (r j), tile n, free (f d)
    x0_v = x0.rearrange("r (j n f) d -> (r j) n (f d)", j=NJ, n=nt, f=ts)
    out_v = out.rearrange("r (j f) -> (r j) f", j=NJ)

    with tile.TileContext(nc) as tc:
        cpool = tc.alloc_tile_pool(name="consts", bufs=1)
        xpool = tc.alloc_tile_pool(name="xin", bufs=xbufs)
        spool = tc.alloc_tile_pool(name="scratch", bufs=sbufs)
        apool = tc.alloc_tile_pool(name="acc", bufs=2)
        ppool = tc.alloc_tile_pool(name="psum", bufs=2, space="PSUM")

        wb_cols = (TS if ((stt_split and stt_split[1]) or gps[0]) else 1) * D
        wb = cpool.tile([128, wb_cols], f32)
        nc.sync.dma_start(out=wb[:], in_=wb_d[:, 0:wb_cols])
        sblk = cpool.tile([128, 128], f32)
        nc.sync.dma_start(out=sblk[:], in_=sblk_d[:])

        def body():
            A = apool.tile([128, FPT], f32, tag="A")
            # ---- matvec: A[p, col] = sum_d x0_tile[p, col, d] * W[d] ----
            split = stt_split or (ts, 0, 0)
            dve_n, act_n, gps_n = split
            assert dve_n + act_n + gps_n == ts
            xt0 = None
            if mode == "compute":
                xt0 = xpool.tile([128, ts * D], f32, tag="xt")
                nc.sync.dma_start(out=xt0[:], in_=x0_v[:, 0, :])
            for n in range(nt):
                if mode == "compute":
                    xt = xt0
                else:
                    xt = xpool.tile([128, ts * D], f32, tag="xt")
                    if dma_ways == 1:
                        nc.sync.dma_start(out=xt[:], in_=x0_v[:, n, :])
                    else:
                        step = 128 // dma_ways
                        for w in range(dma_ways):
                            eng = nc.sync if w % 2 == 0 else nc.scalar
                            eng.dma_start(
                                out=xt[w * step:(w + 1) * step, :],
                                in_=x0_v[w * step:(w + 1) * step, n, :])
                if mode == "dma":
                    continue
                assert xt is not None
                # one big DVE multiply covering the ACT-reduced columns
                if act_n:
                    P = spool.tile([128, act_n * D], f32, tag="P")
                    nc.vector.tensor_tensor(
                        P[:], xt[:, dve_n * D:(dve_n + act_n) * D],
                        wb[:, 0:act_n * D], Alu.mult)
                scbig = spool.tile([128, ts * D], f32, tag="scbig")
                if gps[0] and (n % gps[1]) < gps[0]:
                    # DVE big multiply; GPSIMD log2 halving-add reduce
                    nc.vector.tensor_tensor(
                        scbig[:], xt[:], wb[:, 0:ts * D], Alu.mult)
                    sb3 = scbig[:].rearrange("p (t d) -> p t d", t=ts)
                    w_ = D // 2
                    for lv in range(gps_levels):
                        if w_ > 1:
                            nc.gpsimd.tensor_tensor(
                                sb3[:, :, 0:w_], sb3[:, :, 0:w_],
                                sb3[:, :, w_:2 * w_], Alu.add)
                            w_ //= 2
                    if w_ == 1:
                        nc.gpsimd.tensor_tensor(
                            A[:, n * ts:(n + 1) * ts],
                            sb3[:, :, 0:1].rearrange("p t d -> p (t d)"),
                            sb3[:, :, 1:2].rearrange("p t d -> p (t d)"),
                            Alu.add)
                    else:
                        nc.vector.tensor_reduce(
                            A[:, n * ts:(n + 1) * ts], sb3[:, :, 0:2 * w_],
                            axis=mybir.AxisListType.X, op=Alu.add)
                    continue
                for i in range(ts):
                    col = n * ts + i
                    if i < dve_n:
                        nc.vector.scalar_tensor_tensor(
                            scbig[:, i * D:(i + 1) * D],
                            xt[:, i * D:(i + 1) * D], 1.0, wb[:, 0:D],
                            Alu.mult, Alu.mult,
                            accum_out=A[:, col:col + 1])
                    elif i < dve_n + act_n:
                        j = i - dve_n
                        sc2 = spool.tile([128, D], f32, tag="sc2")
                        nc.scalar.activation(
                            sc2[:], P[:, j * D:(j + 1) * D], Act.Copy,
                            accum_out=A[:, col:col + 1])
                    else:
                        sc = spool.tile([128, D], f32, tag="sc")
                        nc.gpsimd.scalar_tensor_tensor(
                            sc[:], xt[:, i * D:(i + 1) * D], 1.0, wb[:, 0:D],
                            Alu.mult, Alu.mult,
                            accum_out=A[:, col:col + 1])

            if mode == "dma":
                O = apool.tile([128, FPT], f32, tag="O")
                nc.vector.memset(O[:], 0.0)
                nc.sync.dma_start(out=out_v[:, :], in_=O[:])
                return

            # ---- relu ----
            A2 = apool.tile([128, FPT], f32, tag="A2")
            nc.vector.tensor_scalar_max(A2[:], A[:], 0.0)

            if mode == "matvec":
                nc.sync.dma_start(out=out_v[:, :], in_=A2[:])
                return

            # ---- top-k phase A: per-partition top-(8*ka_rounds) ----
            C = apool.tile([128, FPT], f32, tag="C")
            nc.vector.tensor_copy(C[:], A2[:])
            cand = apool.tile([128, 8 * ka_rounds], f32, tag="cand")
            for rnd in range(ka_rounds):
                nc.vector.max(cand[:, 8 * rnd:8 * rnd + 8], C[:])
                if rnd + 1 < ka_rounds:
                    nc.vector.match_replace(
                        C[:], cand[:, 8 * rnd:8 * rnd + 8], C[:], NEG)

            # ---- gather candidates to one row per partition ----
            crow = apool.tile([RPC, NJ * 8 * ka_rounds], f32, tag="crow")
            for r in range(RPC):
                nc.sync.dma_start(
                    out=crow[r:r + 1, :],
                    in_=cand[16 * r:16 * r + 16, :])

            # ---- phase B: exact k-th largest of each row ----
            c8 = apool.tile([RPC, 8], f32, tag="c8")
            for rnd in range(kb_rounds):
                nc.vector.max(c8[:], crow[:])
                nc.vector.match_replace(crow[:], c8[:], crow[:], NEG)
            thr = apool.tile([RPC, 1], f32, tag="thr")
            if kb_rem == 0:
                nc.vector.tensor_reduce(thr[:], crow[:],
                                        axis=mybir.AxisListType.X, op=Alu.max)
            else:
                nc.vector.max(c8[:], crow[:])
                nc.vector.tensor_copy(thr[:], c8[:, kb_rem:kb_rem + 1])

            # ---- broadcast thr back to [128, 1] ----
            ones16 = cpool.tile([RPC, NJ], f32, tag="ones16")
            nc.vector.memset(ones16[:], 1.0)
            thr16 = apool.tile([RPC, NJ], f32, tag="thr16")
            nc.vector.tensor_scalar_mul(thr16[:], ones16[:], thr[:])
            thrp = apool.tile([128, 1], f32, tag="thrp")
            for r in range(RPC):
                nc.sync.dma_start(out=thrp[16 * r:16 * r + 16, :],
                                  in_=thr16[r:r + 1, :])

            # ---- mask, exp, fused multiply + per-partition sum ----
            M = apool.tile([128, FPT], f32, tag="M")
            nc.vector.tensor_scalar(M[:], A2[:], thrp[:, 0:1], None, Alu.is_ge)
            E = apool.tile([128, FPT], f32, tag="E")
            nc.scalar.activation(E[:], A2[:], Act.Exp)
            E2 = apool.tile([128, FPT], f32, tag="E2")
            psum = apool.tile([128, 1], f32, tag="psum")
            nc.vector.scalar_tensor_tensor(
                E2[:], E[:], 1.0, M[:], Alu.mult, Alu.mult,
                accum_out=psum[:])

            # ---- row sums broadcast via block-diagonal matmul ----
            rs = ppool.tile([128, 1], f32, tag="rs")
            nc.tensor.matmul(rs[:], sblk[:], psum[:], start=True, stop=True)
            rinv = apool.tile([128, 1], f32, tag="rinv")
            nc.vector.reciprocal(rinv[:], rs[:])

            # ---- normalize + store ----
            O = apool.tile([128, FPT], f32, tag="O")
            nc.vector.tensor_scalar_mul(O[:], E2[:], rinv[:, 0:1])
            nc.sync.dma_start(out=out_v[:, :], in_=O[:])

        if reps == 1:
            body()
        else:
            with tc.For_i(0, reps, 1):
                body()

        for p in (ppool, apool, spool, xpool, cpool):
            p.release()

    nc.compile()
    return nc


def _consts(W):
    wb = np.ascontiguousarray(
        np.tile(np.asarray(W, np.float32).reshape(1, D), (128, TS)))
    sblk = np.zeros((128, 128), np.float32)
    for r in range(RPC):
        sblk[16 * r:16 * r + 16, 16 * r:16 * r + 16] = 1.0
    return wb, sblk


_CACHE = {}


def kernel(x0, W, k):
    from concourse.bass_utils import run_bass_kernel_spmd

    k = int(np.asarray(k))
    x0 = np.ascontiguousarray(np.asarray(x0, dtype=np.float32))
    assert x0.shape == (B, T, D), x0.shape
    nc = _CACHE.get(k)
    if nc is None:
        nc = _CACHE[k] = build(k)
    wb, sblk = _consts(W)
    in_maps = [
        {"x0": x0[c * RPC:(c + 1) * RPC], "wb": wb, "sblk": sblk}
        for c in range(N_CORES)
    ]
    res = run_bass_kernel_spmd(nc, in_maps, core_ids=list(range(N_CORES)))
    full = np.concatenate([res.results[c]["out"] for c in range(N_CORES)], axis=0)
    return full.reshape(B, T, 1).astype(np.float32)
